# revision 7
# baseline (speedup 1.0000x reference)
"""Stein solver  Lambda - A @ Lambda @ W = C @ Y  on 8 trn2 NeuronCores.

Algorithm: Smith doubling.  S <- S + A_m S W_m;  A_{m+1} = A_m^2, W_{m+1} = W_m^2.
M_DOUBLE doubling steps capture 2^M_DOUBLE terms of Lambda = sum_k A^k R W^k
(contraction ||A||*||W|| ~ 0.32 makes 8 terms sufficient to 1.7e-9), then
N_POLISH fixed-point iterations S <- R + A S W refine to fp32 accuracy.

Distribution: row-sharded over 8 cores.  Core c owns rows [128c, 128c+128).
Per complex GEMM, core c computes its 128-row output block: the stationary
(weights) operand is the transposed own-shard (8 k-tiles of [128,128]); the
moving operand is the full matrix streamed from DRAM.  After each step the
updated shards (S', A', W') are AllGather'ed so every core has full matrices.

Precision: doubling GEMMs run in float32r (1 cyc/row, ~13 mantissa bits);
polish GEMMs run in fp32 (4 cyc/row, exact).  Each polish iteration contracts
the error by the spectral factor ~0.08, so 2 polish iterations land at ~5e-7.

Complex arithmetic: 4 real GEMMs per complex GEMM, with the subtraction of
the real part folded into PSUM accumulation via pre-negated imag weights.
"""

import numpy as np

P = 128
N = 1024
KT = N // P          # 8 k-tiles
NC = 8               # cores
NCH = 2              # 512-wide n-chunks per 1024 output row
M_DOUBLE = 3
N_POLISH = 2

_compiled = {}


def _build():
    import concourse.mybir as mybir
    import concourse.tile as tile
    from concourse import bacc
    from concourse.masks import make_identity

    f32 = mybir.dt.float32
    f32r = mybir.dt.float32r

    nc = bacc.Bacc("TRN2", target_bir_lowering=False, debug=False, num_devices=NC)

    # ---- I/O ----
    # full matrices, layout [partition, ktile, col] per plane: X[kt*128+p, c] at [p, kt, c]
    # f32r-typed data is rounded to ~13 mantissa bits at rest, so the phases
    # that need exact fp32 (RHS cgemm, polish) get separate fp32-typed copies.
    Afull = nc.dram_tensor("Afull", [P, 2, KT, N], f32r, kind="ExternalInput")
    Wfull = nc.dram_tensor("Wfull", [P, 2, KT, N], f32r, kind="ExternalInput")
    Wfull32 = nc.dram_tensor("Wfull32", [P, 2, KT, N], f32, kind="ExternalInput")
    Yfull32 = nc.dram_tensor("Yfull32", [P, 2, KT, N], f32, kind="ExternalInput")
    # transposed own-shards (weights): planes (re, im, -im)
    ATsh = nc.dram_tensor("ATsh", [P, 3, KT, P], f32r, kind="ExternalInput")
    ATsh32 = nc.dram_tensor("ATsh32", [P, 3, KT, P], f32, kind="ExternalInput")
    WTsh = nc.dram_tensor("WTsh", [P, 3, KT, P], f32r, kind="ExternalInput")
    CTsh32 = nc.dram_tensor("CTsh32", [P, 3, KT, P], f32, kind="ExternalInput")
    out = nc.dram_tensor("out", [2, P, N], f32, kind="ExternalOutput")

    RG = [list(range(NC))]

    with tile.TileContext(nc) as tc:
        with (
            tc.tile_pool(name="wpool", bufs=1) as wpool,      # pinned weights
            tc.tile_pool(name="wrot", bufs=2) as wrot,        # rotating weights
            tc.tile_pool(name="rhs", bufs=3) as rpool,        # rhs stream tiles
            tc.tile_pool(name="acc", bufs=2) as apool,        # shard accumulators
            tc.tile_pool(name="stage", bufs=4) as spool,      # psum->dram staging
            tc.tile_pool(name="psum", bufs=4, space="PSUM") as ppool,
            tc.tile_pool(name="tpsum", bufs=2, space="PSUM") as tppool,
            tc.tile_pool(name="dram", bufs=1, space="DRAM") as dram,
        ):
            ident = wpool.tile([P, P], f32, tag="ident")
            make_identity(nc, ident)

            # ---------- helpers ----------
            def load_weights(dram_t, tag, pool=wrot, dtype=f32r):
                wt = pool.tile([P, 3, KT, P], dtype, tag=tag, name="wt_" + tag)
                nc.sync.dma_start(wt[:], dram_t.ap())
                return wt

            def cgemm(XT, rhs_slice, out_cb, dtype=f32r):
                """Complex GEMM: out(128x1024 complex) = own_rows(X) @ M.

                XT: [P,3,KT,P] weight tile (planes re, im, -im), dtype `dtype`.
                rhs_slice(j, t): DRAM AP [P, N] = plane j, k-tile t of M.
                out_cb(j, ci, psum): consume finished [P,512] psum chunk.
                """
                ps = [[ppool.tile([P, 512], f32, tag="ps", name="ps") for _ in range(NCH)]
                      for _ in range(2)]
                for t in range(KT):
                    rt = rpool.tile([P, 2, N], dtype, tag="rhs", name="rt")
                    nc.sync.dma_start(rt[:, 0], rhs_slice(0, t))
                    nc.sync.dma_start(rt[:, 1], rhs_slice(1, t))
                    st = t == 0
                    sp = t == KT - 1
                    for ci in range(NCH):
                        cs = slice(512 * ci, 512 * ci + 512)
                        # re: Xr Yr + (-Xi) Yi ; im: Xr Yi + Xi Yr
                        nc.tensor.matmul(ps[0][ci][:], XT[:, 0, t], rt[:, 0, cs], start=st, stop=False)
                        nc.tensor.matmul(ps[0][ci][:], XT[:, 2, t], rt[:, 1, cs], start=False, stop=sp)
                        nc.tensor.matmul(ps[1][ci][:], XT[:, 0, t], rt[:, 1, cs], start=st, stop=False)
                        nc.tensor.matmul(ps[1][ci][:], XT[:, 1, t], rt[:, 0, cs], start=False, stop=sp)
                for j in range(2):
                    for ci in range(NCH):
                        out_cb(j, ci, ps[j][ci])

            def transpose_to_weights(src, tag, dtype=f32r, pool=wrot):
                """src: [P, 2, N] shard tile -> [P,3,KT,P] transposed weights."""
                wt = pool.tile([P, 3, KT, P], dtype, tag=tag, name="tw_" + tag)
                for j in range(2):
                    for t in range(KT):
                        tp = tppool.tile([P, P], f32, tag="tp", name="tp")
                        nc.tensor.transpose(
                            tp[:], src[:, j, 128 * t:128 * t + 128].bitcast(f32), ident
                        )
                        nc.vector.tensor_copy(wt[:, j, t], tp[:])
                        if j == 1:
                            nc.vector.tensor_scalar_mul(wt[:, 2, t], tp[:], -1.0)
                return wt

            def cb_store(dst):
                """Copyback into [P, 2, N] accumulator tile."""
                def cb(j, ci, psum):
                    nc.vector.tensor_copy(dst[:, j, 512 * ci:512 * ci + 512], psum[:])
                return cb

            def cb_store_add(dst, addend):
                def cb(j, ci, psum):
                    cs = slice(512 * ci, 512 * ci + 512)
                    nc.vector.tensor_add(
                        dst[:, j, cs], psum[:], addend[:, j, cs].bitcast(f32)
                    )
                return cb

            def cb_stage_dma(ag_in, base):
                """Copy psum chunks via staging tiles straight to ag_in[base+j]."""
                def cb(j, ci, psum):
                    stg = spool.tile([P, 512], f32r, tag="stg", name="stg")
                    nc.vector.tensor_copy(stg[:], psum[:])
                    nc.sync.dma_start(
                        ag_in[base + j, :, 512 * ci:512 * ci + 512], stg[:]
                    )
                return cb

            def allgather(ag_in, ag_out):
                nc.gpsimd.collective_compute(
                    "AllGather", mybir.AluOpType.bypass, replica_groups=RG,
                    ins=[ag_in.opt()], outs=[ag_out.opt()],
                )

            # ---------- phase 0: RHS = C @ Y (fp32 for an exact R) ----------
            CTw = load_weights(CTsh32, tag="wA", dtype=f32)
            ag0_in = dram.tile([2, P, N], f32r, name="ag_s_in")
            ag0_out = dram.tile([NC, 2, P, N], f32r, addr_space="Shared", name="ag_s_out")

            S0 = apool.tile([P, 2, N], f32r, tag="Schain", bufs=3, name="Sch")
            R32 = apool.tile([P, 2, N], f32, tag="R32", bufs=1)

            def cb_rhs(j, ci, psum):
                cs = slice(512 * ci, 512 * ci + 512)
                nc.vector.tensor_copy(S0[:, j, cs], psum[:])
                nc.vector.tensor_copy(R32[:, j, cs], psum[:])
                nc.sync.dma_start(ag0_in[j, :, cs], S0[:, j, cs])

            cgemm(CTw, lambda j, t: Yfull32.ap()[:, j, t], cb_rhs, dtype=f32)
            allgather(ag0_in, ag0_out)

            # ---------- doubling steps ----------
            AT32p = load_weights(ATsh32, tag="AT32", pool=wpool, dtype=f32)  # pinned for polish
            ATw = load_weights(ATsh, tag="wA")
            WTw = load_weights(WTsh, tag="wW")

            S_own = S0
            S_src = lambda j, t: ag0_out[t, j]               # noqa: E731
            A_src = lambda j, t: Afull.ap()[:, j, t]         # noqa: E731
            W_src = lambda j, t: Wfull.ap()[:, j, t]         # noqa: E731
            ATcur, WTcur = ATw, WTw

            for m in range(M_DOUBLE):
                last = m == M_DOUBLE - 1
                nplanes = 2 if last else 6
                ag_in = dram.tile([nplanes, P, N], f32r, name="ag_in")
                ag_out = dram.tile([NC, nplanes, P, N], f32r, addr_space="Shared", name="ag_out")

                # U = A_m @ S_m  (own rows)
                U = apool.tile([P, 2, N], f32r, tag="U", name="U")
                cgemm(ATcur, S_src, cb_store(U))
                UTw = transpose_to_weights(U, tag="UT")

                # S' = S + U @ W_m (own rows) -> ag_in[0:2] and, if last, keep S3
                Snew = apool.tile([P, 2, N], f32r, tag="Schain", bufs=3, name="Sch")

                def cb_snew(j, ci, psum, Snew=Snew, S_own=S_own, ag_in=ag_in):
                    cs = slice(512 * ci, 512 * ci + 512)
                    nc.vector.tensor_add(
                        Snew[:, j, cs], psum[:], S_own[:, j, cs].bitcast(f32)
                    )
                    nc.sync.dma_start(ag_in[j, :, cs], Snew[:, j, cs])

                cgemm(UTw, W_src, cb_snew)

                if not last:
                    # A' = A_m^2, W' = W_m^2 (own rows)
                    Anew = apool.tile([P, 2, N], f32r, tag="Asq", name="Asq")
                    cgemm(ATcur, A_src, cb_store(Anew))
                    for j in range(2):
                        nc.sync.dma_start(ag_in[2 + j], Anew[:, j])
                    Wnew = apool.tile([P, 2, N], f32r, tag="Wsq", name="Wsq")
                    cgemm(WTcur, W_src, cb_store(Wnew))
                    for j in range(2):
                        nc.sync.dma_start(ag_in[4 + j], Wnew[:, j])
                    ATcur = transpose_to_weights(Anew, tag="wA")
                    WTcur = transpose_to_weights(Wnew, tag="wW")

                allgather(ag_in, ag_out)
                S_own = Snew
                S_src = lambda j, t, ag_out=ag_out: ag_out[t, j]          # noqa: E731
                if not last:
                    A_src = lambda j, t, ag_out=ag_out: ag_out[t, 2 + j]  # noqa: E731
                    W_src = lambda j, t, ag_out=ag_out: ag_out[t, 4 + j]  # noqa: E731

            # ---------- polish iterations (fp32) ----------
            AT32 = AT32p
            for it in range(N_POLISH):
                last = it == N_POLISH - 1
                # U = A @ S (fp32)
                U32 = apool.tile([P, 2, N], f32, tag="U", name="U32")
                cgemm(AT32, lambda j, t: S_src(j, t).bitcast(f32),
                      cb_store(U32), dtype=f32)
                UT32 = transpose_to_weights(U32, tag="UT", dtype=f32)

                Sp = apool.tile([P, 2, N], f32, tag="Schain", bufs=3, name="Sch")
                if not last:
                    ag_in = dram.tile([2, P, N], f32, name="agp_in")
                    ag_out = dram.tile([NC, 2, P, N], f32, addr_space="Shared", name="agp_out")

                    def cb_sp(j, ci, psum, Sp=Sp, ag_in=ag_in):
                        cs = slice(512 * ci, 512 * ci + 512)
                        nc.vector.tensor_add(Sp[:, j, cs], psum[:], R32[:, j, cs])
                        nc.sync.dma_start(ag_in[j, :, cs], Sp[:, j, cs])

                    cgemm(UT32, lambda j, t: Wfull32.ap()[:, j, t],
                          cb_sp, dtype=f32)
                    allgather(ag_in, ag_out)
                    S_src = lambda j, t, ag_out=ag_out: ag_out[t, j]  # noqa: E731
                else:
                    def cb_fin(j, ci, psum, Sp=Sp):
                        cs = slice(512 * ci, 512 * ci + 512)
                        nc.vector.tensor_add(Sp[:, j, cs], psum[:], R32[:, j, cs])
                        nc.sync.dma_start(out.ap()[j, :, cs], Sp[:, j, cs])

                    cgemm(UT32, lambda j, t: Wfull32.ap()[:, j, t],
                          cb_fin, dtype=f32)

    nc.compile()
    return nc


def _prep_inputs(A, W, C, Y):
    """Host-side: fp32 planes in device layouts, per core."""
    def full_layout(M):
        # [2, P, KT, N] -> stored as [P, 2, KT, N]
        pl = np.stack([
            np.ascontiguousarray(M.real.astype(np.float32)),
            np.ascontiguousarray(M.imag.astype(np.float32)),
        ])  # [2, 1024, 1024]
        return np.ascontiguousarray(
            pl.reshape(2, KT, P, N).transpose(2, 0, 1, 3)
        )  # [P, 2, KT, N]

    def shard_weights(M, c):
        own = M[P * c:P * c + P, :]           # [128, 1024]
        XT = own.T                            # [1024, 128]
        r = XT.real.astype(np.float32)
        i = XT.imag.astype(np.float32)
        tr = np.stack([r, i, -i])             # [3, 1024, 128]
        return np.ascontiguousarray(
            tr.reshape(3, KT, P, P).transpose(2, 0, 1, 3)
        )  # [P, 3, KT, P]

    Af, Wf, Yf = full_layout(A), full_layout(W), full_layout(Y)
    in_maps = []
    for c in range(NC):
        ATs = shard_weights(A, c)
        in_maps.append({
            "Afull": Af, "Wfull": Wf, "Wfull32": Wf, "Yfull32": Yf,
            "ATsh": ATs, "ATsh32": ATs,
            "WTsh": shard_weights(W, c),
            "CTsh32": shard_weights(C, c),
        })
    return in_maps


def kernel(A, W, C, Y, _trace=False):
    from concourse import bass_utils

    if "nc" not in _compiled:
        _compiled["nc"] = _build()
    nc = _compiled["nc"]

    in_maps = _prep_inputs(A, W, C, Y)
    res = bass_utils.run_bass_kernel_spmd(
        nc, in_maps, core_ids=list(range(NC)), trace=_trace
    )
    _compiled["last_result"] = res

    full = np.empty((N, N), dtype=np.complex128)
    for c in range(NC):
        o = res.results[c]["out"]
        full[P * c:P * c + P, :] = o[0].astype(np.float64) + 1j * o[1].astype(np.float64)
    return full


# revision 11
# speedup vs baseline: 1.6071x; 1.6071x over previous
"""Stein solver  Lambda - A @ Lambda @ W = C @ Y  on 8 trn2 NeuronCores.

Math: Lambda = sum_k A^k R W^k with R = C@Y; contraction ||A||2*||W||2 ~ 0.32.
Smith doubling truncated at 4 terms, then 2 exact fixed-point iterations:
    S1 = R + (A C) Y W           (2 terms; U0 = (A C) Y folds R's producer in)
    S2 = S1 + A^2 S1 W^2         (4 terms)
    Sp = R + A S W   (x2, fp32)  (polish: each contracts error ~12x -> ~1e-6)

Distribution: row-sharded over 8 cores, core c owns rows [128c, 128c+128).
Stationary operand = transposed own-shard (8 k-tiles of [128,128]); moving
operand = full matrix streamed from DRAM.  AllGathers carry only what later
phases read as full matrices: S1 (bf16), W2 (bf16), S2 (f32r), Sp (fp32) --
bf16 suffices wherever the consumer term is ~1% of S and later polished.

Precision tiers per complex GEMM (4 real GEMMs, real-part subtraction folded
into PSUM accumulation via pre-negated imag weights):
  fp32  (4 cyc/row): RHS, polish -- enter the answer directly.
  f32r  (1 cyc/row, ~13 mantissa bits): V=A@C, U0, S1 -- pre-polish chain.
  bf16  (1 cyc/row, half DMA): A^2, W^2, U1, S2's product -- 1%-scale terms.
"""

import numpy as np

P = 128
N = 1024
KT = N // P          # 8 k-tiles
NC = 8               # cores
NCH = 2              # 512-wide n-chunks per 1024-col output row block

_compiled = {}


def _build():
    import concourse.mybir as mybir
    import concourse.tile as tile
    from concourse import bacc
    from concourse.masks import make_identity

    f32 = mybir.dt.float32
    f32r = mybir.dt.float32r
    bf16 = mybir.dt.bfloat16

    nc = bacc.Bacc("TRN2", target_bir_lowering=False, debug=False, num_devices=NC)

    # ---- I/O ----  full matrices laid out [partition, plane, ktile, col]:
    # X[kt*128+p, c] at [p, j, kt, c]; shards [partition, (re,im,-im), ktile, m]
    Cfull = nc.dram_tensor("Cfull", [P, 2, KT, N], f32r, kind="ExternalInput")
    Yfull32 = nc.dram_tensor("Yfull32", [P, 2, KT, N], f32, kind="ExternalInput")
    Wfull = nc.dram_tensor("Wfull", [P, 2, KT, N], f32r, kind="ExternalInput")
    Wfull32 = nc.dram_tensor("Wfull32", [P, 2, KT, N], f32, kind="ExternalInput")
    Afull_bf = nc.dram_tensor("Afull_bf", [P, 2, KT, N], bf16, kind="ExternalInput")
    Wfull_bf = nc.dram_tensor("Wfull_bf", [P, 2, KT, N], bf16, kind="ExternalInput")
    ATsh = nc.dram_tensor("ATsh", [P, 3, KT, P], f32r, kind="ExternalInput")
    ATsh32 = nc.dram_tensor("ATsh32", [P, 3, KT, P], f32, kind="ExternalInput")
    ATsh_bf = nc.dram_tensor("ATsh_bf", [P, 3, KT, P], bf16, kind="ExternalInput")
    WTsh_bf = nc.dram_tensor("WTsh_bf", [P, 3, KT, P], bf16, kind="ExternalInput")
    CTsh32 = nc.dram_tensor("CTsh32", [P, 3, KT, P], f32, kind="ExternalInput")
    out = nc.dram_tensor("out", [2, P, N], f32, kind="ExternalOutput")

    RG = [list(range(NC))]

    with tile.TileContext(nc) as tc:
        with (
            tc.tile_pool(name="wpin", bufs=1) as wpin,        # pinned
            tc.tile_pool(name="wrot", bufs=2) as wrot,        # rotating weights
            tc.tile_pool(name="rhs", bufs=3) as rpool,        # rhs stream tiles
            tc.tile_pool(name="acc", bufs=2) as apool,        # shard accumulators
            tc.tile_pool(name="psum", bufs=6, space="PSUM") as ppool,
            tc.tile_pool(name="tpsum", bufs=2, space="PSUM") as tppool,
            tc.tile_pool(name="dram", bufs=1, space="DRAM") as dram,
        ):
            ident = wpin.tile([P, P], f32, tag="ident")
            make_identity(nc, ident)
            ident_bf = wpin.tile([P, P], bf16, tag="identbf")
            nc.vector.tensor_copy(ident_bf[:], ident[:])

            def load_weights(dram_t, tag, dtype, pool=wrot):
                wt = pool.tile([P, 3, KT, P], dtype, tag=tag, name="wt_" + tag)
                nc.sync.dma_start(wt[:], dram_t.ap())
                return wt

            def cgemm(XT, rhs_slice, out_cb, dtype):
                """out(own 128 rows x 1024, complex) = own_rows(X) @ M.

                XT: [P,3,KT,P] weights (re, im, -im); rhs_slice(j,t) -> DRAM
                [P,N] AP of plane j, k-tile t of M; out_cb(j, ci, psum).
                """
                ps = [[ppool.tile([P, 512], f32, tag="ps", name="ps")
                       for _ in range(NCH)] for _ in range(2)]
                for t in range(KT):
                    rt = rpool.tile([P, 2, N], dtype, tag="rhs", name="rt")
                    nc.sync.dma_start(rt[:, 0], rhs_slice(0, t))
                    nc.sync.dma_start(rt[:, 1], rhs_slice(1, t))
                    st = t == 0
                    sp = t == KT - 1
                    for ci in range(NCH):
                        cs = slice(512 * ci, 512 * ci + 512)
                        nc.tensor.matmul(ps[0][ci][:], XT[:, 0, t], rt[:, 0, cs], start=st, stop=False)
                        nc.tensor.matmul(ps[0][ci][:], XT[:, 2, t], rt[:, 1, cs], start=False, stop=sp)
                        nc.tensor.matmul(ps[1][ci][:], XT[:, 0, t], rt[:, 1, cs], start=st, stop=False)
                        nc.tensor.matmul(ps[1][ci][:], XT[:, 1, t], rt[:, 0, cs], start=False, stop=sp)
                for j in range(2):
                    for ci in range(NCH):
                        out_cb(j, ci, ps[j][ci])

            def transpose_to_weights(src, tag, dtype, pool=wrot):
                """[P, 2, N] shard tile -> [P,3,KT,P] transposed weights."""
                wt = pool.tile([P, 3, KT, P], dtype, tag=tag, name="tw_" + tag)
                bf = src.dtype == bf16
                for j in range(2):
                    for t in range(KT):
                        tp = tppool.tile([P, P], bf16 if bf else f32, tag="tp", name="tp")
                        blk = src[:, j, 128 * t:128 * t + 128]
                        if bf:
                            nc.tensor.transpose(tp[:], blk, ident_bf[:])
                        else:
                            nc.tensor.transpose(tp[:], blk.bitcast(f32), ident[:])
                        nc.vector.tensor_copy(wt[:, j, t], tp[:])
                        if j == 1:
                            nc.vector.tensor_scalar_mul(wt[:, 2, t], tp[:], -1.0)
                return wt

            def cb_store(dst):
                def cb(j, ci, psum):
                    nc.vector.tensor_copy(dst[:, j, 512 * ci:512 * ci + 512], psum[:])
                return cb

            def allgather(ag_in, ag_out):
                nc.gpsimd.collective_compute(
                    "AllGather", mybir.AluOpType.bypass, replica_groups=RG,
                    ins=[ag_in.opt()], outs=[ag_out.opt()],
                )

            def src_of(dram_t, base=0):
                ap = dram_t.ap() if hasattr(dram_t, "ap") else dram_t
                return lambda j, t: ap[:, base + j, t]

            # ---------------- phase 1 (no collective deps) ----------------
            # V = A @ C   (f32r)
            ATw = load_weights(ATsh, tag="T1", dtype=f32r)
            CT32 = load_weights(CTsh32, tag="T3", dtype=f32)
            V = apool.tile([P, 2, N], f32r, tag="work", bufs=4, name="V")
            cgemm(ATw, src_of(Cfull), cb_store(V), f32r)

            # R = C @ Y   (fp32, exact; kept for the polish additive term)
            R32 = apool.tile([P, 2, N], f32, tag="R32", bufs=1)
            cgemm(CT32, src_of(Yfull32), cb_store(R32), f32)

            VT = transpose_to_weights(V, tag="T3", dtype=f32r)

            # U0 = V @ Y  (f32r; rhs bytes are full fp32, PE rounds on read)
            U0 = apool.tile([P, 2, N], f32r, tag="work", bufs=4, name="U0")
            ysrc = src_of(Yfull32)
            cgemm(VT, lambda j, t: ysrc(j, t).bitcast(f32r), cb_store(U0), f32r)
            U0T = transpose_to_weights(U0, tag="T1", dtype=f32r)

            # S1 = R + U0 @ W  (f32r chain value; bf16 copy for the AllGather)
            S1 = apool.tile([P, 2, N], f32r, tag="Sch", bufs=2, name="S1")
            agb_in = dram.tile([2, P, N], bf16, name="agb_in")
            agb_out = dram.tile([NC, 2, P, N], bf16, addr_space="Shared", name="agb_out")

            def cb_s1(j, ci, psum):
                cs = slice(512 * ci, 512 * ci + 512)
                nc.vector.tensor_add(S1[:, j, cs], psum[:], R32[:, j, cs])
                stg = apool.tile([P, 512], bf16, tag="stg", bufs=4, name="stg")
                nc.vector.tensor_add(stg[:], psum[:], R32[:, j, cs])
                nc.sync.dma_start(agb_in[j, :, cs], stg[:])

            cgemm(U0T, src_of(Wfull), cb_s1, f32r)
            allgather(agb_in, agb_out)

            # A1 = A^2, W1 = W^2  (bf16; fills the AG window)
            ATbf = load_weights(ATsh_bf, tag="T2", dtype=bf16)
            WTbf = load_weights(WTsh_bf, tag="T3", dtype=bf16)
            A1 = apool.tile([P, 2, N], bf16, tag="work", bufs=4, name="A1")
            cgemm(ATbf, src_of(Afull_bf), cb_store(A1), bf16)
            AT1 = transpose_to_weights(A1, tag="T2", dtype=bf16)

            aga_in = dram.tile([2, P, N], bf16, name="aga_in")
            aga_out = dram.tile([NC, 2, P, N], bf16, addr_space="Shared", name="aga_out")
            W1 = apool.tile([P, 2, N], bf16, tag="work", bufs=4, name="W1")

            def cb_w1(j, ci, psum):
                cs = slice(512 * ci, 512 * ci + 512)
                nc.vector.tensor_copy(W1[:, j, cs], psum[:])
                nc.sync.dma_start(aga_in[j, :, cs], W1[:, j, cs])

            cgemm(WTbf, src_of(Wfull_bf), cb_w1, bf16)
            allgather(aga_in, aga_out)

            # ---------------- step 2: S2 = S1 + A1 S1 W1 (bf16 GEMMs) -----
            U1 = apool.tile([P, 2, N], bf16, tag="work", bufs=4, name="U1")
            cgemm(AT1, lambda j, t: agb_out[t, j], cb_store(U1), bf16)
            U1T = transpose_to_weights(U1, tag="T1", dtype=bf16)

            S2 = apool.tile([P, 2, N], f32r, tag="Sch", bufs=2, name="S2")
            agc_in = dram.tile([2, P, N], f32r, name="agc_in")
            agc_out = dram.tile([NC, 2, P, N], f32r, addr_space="Shared", name="agc_out")

            def cb_s2(j, ci, psum):
                cs = slice(512 * ci, 512 * ci + 512)
                nc.vector.tensor_add(S2[:, j, cs], psum[:], S1[:, j, cs].bitcast(f32))
                nc.sync.dma_start(agc_in[j, :, cs], S2[:, j, cs])

            cgemm(U1T, lambda j, t: aga_out[t, j], cb_s2, bf16)
            allgather(agc_in, agc_out)

            # ---------------- polish x2 (fp32) ----------------------------
            AT32 = load_weights(ATsh32, tag="AT32", dtype=f32, pool=wpin)
            s_src = (lambda j, t: agc_out[t, j])
            s_cast = f32r
            for it in range(2):
                last = it == 1
                Up = apool.tile([P, 2, N], f32, tag="work", bufs=4, name="Up")
                if s_cast is not None:
                    cgemm(AT32, lambda j, t: s_src(j, t).bitcast(f32), cb_store(Up), f32)
                else:
                    cgemm(AT32, s_src, cb_store(Up), f32)
                UpT = transpose_to_weights(Up, tag="T2", dtype=f32)

                Sp = apool.tile([P, 2, N], f32, tag="Sch", bufs=2, name="Sp")
                if not last:
                    agd_in = dram.tile([2, P, N], f32, name="agd_in")
                    agd_out = dram.tile([NC, 2, P, N], f32, addr_space="Shared", name="agd_out")

                    def cb_sp(j, ci, psum, Sp=Sp, agd_in=agd_in):
                        cs = slice(512 * ci, 512 * ci + 512)
                        nc.vector.tensor_add(Sp[:, j, cs], psum[:], R32[:, j, cs])
                        nc.sync.dma_start(agd_in[j, :, cs], Sp[:, j, cs])

                    cgemm(UpT, src_of(Wfull32), cb_sp, f32)
                    allgather(agd_in, agd_out)
                    s_src = (lambda j, t, agd_out=agd_out: agd_out[t, j])
                    s_cast = None
                else:
                    def cb_fin(j, ci, psum):
                        cs = slice(512 * ci, 512 * ci + 512)
                        nc.vector.tensor_add(Sp[:, j, cs], psum[:], R32[:, j, cs])
                        nc.sync.dma_start(out.ap()[j, :, cs], Sp[:, j, cs])

                    cgemm(UpT, src_of(Wfull32), cb_fin, f32)

    nc.compile()
    return nc


def _prep_inputs(A, W, C, Y):
    import ml_dtypes
    bf = ml_dtypes.bfloat16

    def full_layout(M, dt=np.float32):
        pl = np.stack([
            M.real.astype(np.float32).astype(dt),
            M.imag.astype(np.float32).astype(dt),
        ])  # [2, 1024, 1024]
        return np.ascontiguousarray(pl.reshape(2, KT, P, N).transpose(2, 0, 1, 3))

    def shard_weights(M, c, dt=np.float32):
        own = M[P * c:P * c + P, :]
        XT = own.T
        r = XT.real.astype(np.float32)
        i = XT.imag.astype(np.float32)
        tr = np.stack([r, i, -i]).astype(dt)  # [3, 1024, 128]
        return np.ascontiguousarray(tr.reshape(3, KT, P, P).transpose(2, 0, 1, 3))

    Cf = full_layout(C)
    Yf = full_layout(Y)
    Wf = full_layout(W)
    Abf = full_layout(A, bf)
    Wbf = full_layout(W, bf)
    in_maps = []
    for c in range(NC):
        ATs = shard_weights(A, c)
        in_maps.append({
            "Cfull": Cf, "Yfull32": Yf, "Wfull": Wf, "Wfull32": Wf,
            "Afull_bf": Abf, "Wfull_bf": Wbf,
            "ATsh": ATs, "ATsh32": ATs,
            "ATsh_bf": shard_weights(A, c, bf),
            "WTsh_bf": shard_weights(W, c, bf),
            "CTsh32": shard_weights(C, c),
        })
    return in_maps


def kernel(A, W, C, Y, _trace=False):
    from concourse import bass_utils

    if "nc" not in _compiled:
        _compiled["nc"] = _build()
    nc = _compiled["nc"]

    in_maps = _prep_inputs(A, W, C, Y)
    res = bass_utils.run_bass_kernel_spmd(
        nc, in_maps, core_ids=list(range(NC)), trace=_trace
    )
    _compiled["last_result"] = res

    full = np.empty((N, N), dtype=np.complex128)
    for c in range(NC):
        o = res.results[c]["out"]
        full[P * c:P * c + P, :] = o[0].astype(np.float64) + 1j * o[1].astype(np.float64)
    return full
